# revision 2
# baseline (speedup 1.0000x reference)
"""TreeLSTM (nn_BinaryTreeLSTM, S=128 B=512 D=256) Trainium2 kernel, v2.

8-core data-parallel over the batch: each NeuronCore owns 64 batch rows and
runs the full 128-step sequential scan locally (no cross-core comms). Host
converts the one-hot child masks to indices, lays tensors out
feature-on-partition, runs one SPMD NEFF on cores 0-7 via
bass_utils.run_bass_kernel_spmd, and reassembles the [512, 128, 256] output.

Math (identical to the reference):
  xp[s] = x[s] @ Wx[g].T + bx                    (4 gates, state-independent)
  per step s:  lh/lc, rh/rc = (h/c)[b, li[s,b]], (h/c)[b, ri[s,b]]
    gates = concat(lh, rh) @ W2 + xp[s]          (W2 = [Wlh; Wrh])
    u = tanh(.); i, lf, rf, o = sigmoid(.)       (lf/rf share the xp f-term)
    cn = i*u + lf*lc + rf*rc; hn = o*tanh(cn)
    h[b, s] = m*hn; c[b, s] = m*cn               (rows start at zero)

v2 design (vs v1): everything stays on-chip.
  - xp is computed on the fly per 2-step group straight INTO the gate PSUM
    tile ([128, 10 chunks, 128 cols], 3 banks, double buffered): 3-term fp16
    compensation (xh@Wh + xl@Wh + xh@Wl) for fp32-accurate xp, plus the bias
    as a rank-1 matmul (bx ox ones). The recurrent gate matmuls then
    accumulate into the same PSUM regions, so the per-gate epilogue is just
    one activation read from PSUM (no DVE adds, no xp DRAM round-trip).
  - x streams from DRAM in 2-step blocks (512B/partition), prefetched.
  - One merged h-gather per step (left+right, 128 idxs, fp16 d=2) and one
    c-gather (f32 d=2) on gpsimd, with prefix num_elems=s*64 (indices only
    ever reference earlier rows). c-gather is emitted right after the c
    writeback so it overlaps the h tail of the previous step.
  - State h fp16 / c fp32 in SBUF; mask folded into the writebacks
    (rows start zeroed, so the blend is a pure multiply).
  - Output is st_h itself: a single 4MB DMA at the end; host restores layout.

Precision: fp32 everywhere except fp16 matmul operands/state-h and the fp16
output; measured fro-rel ~3.5e-4 (same as v1).
"""

import numpy as np

import concourse.bass as bass
import concourse.mybir as mybir
import concourse.tile as tile
from concourse import bacc
from concourse import bass_utils

S, B, D = 128, 512, 256
NCORES = 8
BS = B // NCORES          # 64 batch rows per core
NE = S * BS               # 8192 state rows per core
NMC = 10                  # gate psum chunks (u,i,lf,rf,o) x 2
NG = S // 2               # 2-step groups
# psum gate chunk -> xp weight chunk (rf reuses the f projection)
XP_MAP10 = [0, 1, 2, 3, 4, 5, 4, 5, 6, 7]

F16 = mybir.dt.float16
F32 = mybir.dt.float32
I16 = mybir.dt.int16
AF = mybir.ActivationFunctionType
OP = mybir.AluOpType

_CACHED = {}


def build_program():
    """Trace + compile the per-core Bass program (same NEFF on all 8 cores)."""
    nc = bacc.Bacc("TRN2", target_bir_lowering=False, debug=False)

    d_xh = nc.dram_tensor("xgh", [NG, 128, 2, 128], F16, kind="ExternalInput").ap()
    d_xl = nc.dram_tensor("xgl", [NG, 128, 2, 128], F16, kind="ExternalInput").ap()
    d_w2 = nc.dram_tensor("w2", [128, 4 * NMC, 128], F16, kind="ExternalInput").ap()
    d_wxh = nc.dram_tensor("wxh", [128, 16, 128], F16, kind="ExternalInput").ap()
    d_wxl = nc.dram_tensor("wxl", [128, 16, 128], F16, kind="ExternalInput").ap()
    d_bx = nc.dram_tensor("bx10", [1, NMC, 128], F16, kind="ExternalInput").ap()
    d_idx = nc.dram_tensor("idx", [128, S, 8], I16, kind="ExternalInput").ap()
    d_mask = nc.dram_tensor("maskv", [1, NE], F16, kind="ExternalInput").ap()
    d_out = nc.dram_tensor("hT", [128, NE, 2], F16, kind="ExternalOutput").ap()

    with tile.TileContext(nc) as tc:
        with (
            tc.tile_pool(name="persist", bufs=1) as persist,
            tc.tile_pool(name="xb", bufs=4) as xb,
            tc.tile_pool(name="ps", bufs=2, space="PSUM") as ps,
            tc.tile_pool(name="gpool", bufs=2) as gpool,
            tc.tile_pool(name="gate", bufs=2) as gate,
        ):
            st_h = persist.tile([128, NE, 2], F16)
            st_c = persist.tile([128, NE, 2], F32)
            s_w2 = persist.tile([128, 4 * NMC, 128], F16)
            s_wxh = persist.tile([128, 16, 128], F16)
            s_wxl = persist.tile([128, 16, 128], F16)
            s_bx = persist.tile([1, NMC, 128], F16)
            s_ones = persist.tile([1, 128], F16)
            s_idx = persist.tile([128, S, 8], I16)
            s_mask = persist.tile([128, NE], F16)

            nc.sync.dma_start(out=s_w2[:], in_=d_w2[:])
            nc.sync.dma_start(out=s_wxh[:], in_=d_wxh[:])
            nc.sync.dma_start(out=s_wxl[:], in_=d_wxl[:])
            nc.sync.dma_start(out=s_bx[:], in_=d_bx[:])
            nc.sync.dma_start(out=s_idx[:], in_=d_idx[:])
            mask_bcast = bass.AP(
                tensor=d_mask.tensor,
                offset=d_mask.offset,
                ap=[[0, 128]] + list(d_mask.ap[1:]),
            )
            nc.gpsimd.dma_start(out=s_mask[:], in_=mask_bcast)
            nc.vector.memset(s_ones[:], 1.0)
            # only step 0's gather reads unwritten rows; every other row is
            # written (mask-multiplied full write) before any gather reads it
            nc.vector.memset(st_h[:, 0:BS, :], 0.0)
            nc.vector.memset(st_c[:, 0:BS, :], 0.0)

            xp_tiles = {}

            def emit_xp(g):
                """xp for steps (2g, 2g+1) into a [128, NMC, 128] psum tile.

                3 fp16 terms + rank-1 bias accumulate per chunk; start=True
                only on the first matmul touching each 2KB zero region (the
                whole bank is lazily zeroed, later first-touches overwrite).
                """
                xh = xb.tile([128, 2, 128], F16, name=f"xh{g}", tag="xh")
                xl = xb.tile([128, 2, 128], F16, name=f"xl{g}", tag="xl")
                nc.sync.dma_start(out=xh[:], in_=d_xh[g])
                nc.sync.dma_start(out=xl[:], in_=d_xl[g])
                pst = ps.tile([128, NMC, 128], F32, name=f"xp{g}", tag="xp")
                xp_tiles[g] = pst
                started = set()
                for pc in range(NMC):
                    mc = XP_MAP10[pc]
                    bank = pc // 4
                    for kc in range(2):
                        for wmat, xmat in ((s_wxh, xh), (s_wxl, xh), (s_wxh, xl)):
                            first = bank not in started
                            started.add(bank)
                            nc.tensor.matmul(
                                pst[:, pc, :],
                                lhsT=wmat[:, mc * 2 + kc, :],
                                rhs=xmat[:, kc, :],
                                start=first, stop=False,
                            )
                    nc.tensor.matmul(
                        pst[:, pc, :], lhsT=s_bx[:, pc, :], rhs=s_ones[:],
                        start=False, stop=False,
                    )

            def emit_gathers(s):
                """child-state gathers for step s (c first: its writeback
                lands earlier, so the c-gather overlaps the h tail)."""
                pe = max(s, 1) * BS
                gc_t = gpool.tile([128, 2 * BS, 2], F32, name=f"gc{s}", tag="gc")
                nc.gpsimd.ap_gather(
                    gc_t[:], st_c[:, :pe, :], s_idx[:, s, :],
                    channels=128, num_elems=pe, d=2, num_idxs=2 * BS,
                )
                gh_t = gpool.tile([128, 2 * BS, 2], F16, name=f"gh{s}", tag="gh")
                nc.gpsimd.ap_gather(
                    gh_t[:], st_h[:, :pe, :], s_idx[:, s, :],
                    channels=128, num_elems=pe, d=2, num_idxs=2 * BS,
                )
                return gh_t, gc_t

            emit_xp(0)
            gathers = emit_gathers(0)

            for s in range(S):
                g, p01 = divmod(s, 2)
                if p01 == 0 and g + 1 < NG:
                    emit_xp(g + 1)
                pst = xp_tiles[g]
                gh_t, gc_t = gathers
                col = slice(p01 * BS, (p01 + 1) * BS)

                # recurrent gate matmuls accumulate onto the xp psum.
                # stop=True on the last matmul per 2KB psum bank (p01==1).
                for gi in range(5):          # u, i, lf, rf, o
                    for mc2 in range(2):
                        pc = gi * 2 + mc2
                        for kc in range(4):  # (lr, dhi)
                            lr, dhi = divmod(kc, 2)
                            last = (kc == 3 and p01 == 1 and pc in (3, 7, 9))
                            nc.tensor.matmul(
                                pst[:, pc, col],
                                lhsT=s_w2[:, pc * 4 + kc, :],
                                rhs=gh_t[:, lr * BS:(lr + 1) * BS, dhi],
                                start=False, stop=last,
                            )

                u_t = gate.tile([128, 2, BS], F32, name=f"u{s}", tag="u")
                i_t = gate.tile([128, 2, BS], F32, name=f"i{s}", tag="i")
                f_t = gate.tile([128, 2, 2, BS], F32, name=f"f{s}", tag="f")
                o_t = gate.tile([128, 2, BS], F32, name=f"o{s}", tag="o")
                nc.scalar.activation(u_t[:], pst[:, 0:2, col], AF.Tanh)
                nc.scalar.activation(i_t[:], pst[:, 2:4, col], AF.Sigmoid)
                nc.scalar.activation(f_t[:], pst[:, 4:8, col], AF.Sigmoid)
                nc.scalar.activation(o_t[:], pst[:, 8:10, col], AF.Sigmoid)

                t_iu = gate.tile([128, 2, BS], F32, name=f"tiu{s}", tag="tiu")
                t23 = gate.tile([128, 2, 2, BS], F32, name=f"t23{s}", tag="t23")
                cn1 = gate.tile([128, 2, BS], F32, name=f"cn1{s}", tag="cn1")
                cn = gate.tile([128, 2, BS], F32, name=f"cn{s}", tag="cn")
                t_cn = gate.tile([128, 2, BS], F32, name=f"tcn{s}", tag="tcn")
                o_m = gate.tile([128, 2, BS], F32, name=f"om{s}", tag="om")

                nc.vector.tensor_mul(t_iu[:], u_t[:], i_t[:])
                # f_t is [p][(lf,rf)][c][b]; arrange gathered c to match
                cc = gc_t[:].rearrange("p (l i) c -> p l c i", l=2)
                nc.vector.tensor_tensor(out=t23[:], in0=f_t[:], in1=cc, op=OP.mult)
                nc.vector.tensor_add(cn1[:], t_iu[:], t23[:, 0])
                nc.vector.tensor_add(cn[:], cn1[:], t23[:, 1])

                mrow = s_mask[:, s * BS:(s + 1) * BS]
                mb = bass.AP(
                    tensor=mrow.tensor,
                    offset=mrow.offset,
                    ap=[mrow.ap[0], [0, 2]] + list(mrow.ap[1:]),
                )
                rows = slice(s * BS, (s + 1) * BS)
                nc.vector.tensor_tensor(
                    out=st_c[:, rows, :].rearrange("p i c -> p c i"),
                    in0=cn[:], in1=mb, op=OP.mult,
                )
                if s + 1 < S:
                    pe = (s + 1) * BS
                    gc_n = gpool.tile([128, 2 * BS, 2], F32, name=f"gc{s+1}", tag="gc")
                    nc.gpsimd.ap_gather(
                        gc_n[:], st_c[:, :pe, :], s_idx[:, s + 1, :],
                        channels=128, num_elems=pe, d=2, num_idxs=2 * BS,
                    )

                nc.scalar.activation(t_cn[:], cn[:], AF.Tanh)
                nc.vector.tensor_tensor(out=o_m[:], in0=o_t[:], in1=mb, op=OP.mult)
                nc.vector.tensor_tensor(
                    out=st_h[:, rows, :].rearrange("p i c -> p c i"),
                    in0=o_m[:], in1=t_cn[:], op=OP.mult,
                )
                if s + 1 < S:
                    gh_n = gpool.tile([128, 2 * BS, 2], F16, name=f"gh{s+1}", tag="gh")
                    nc.gpsimd.ap_gather(
                        gh_n[:], st_h[:, :pe, :], s_idx[:, s + 1, :],
                        channels=128, num_elems=pe, d=2, num_idxs=2 * BS,
                    )
                    gathers = (gh_n, gc_n)

            nc.sync.dma_start(out=d_out[:], in_=st_h[:])
    nc.compile()
    return nc


def _prep_core_inputs(x, x_mask, li, ri, Wx, bx, Wlh, Wrh, core):
    b0 = core * BS

    xr = x[:, b0:b0 + BS, :]                          # [S, BS, D]
    xT = np.ascontiguousarray(xr.transpose(2, 0, 1))  # [D, S, BS]
    xT = xT.reshape(2, 128, NE).transpose(1, 0, 2)    # [128, 2, NE]; d = dhi*128+p
    xT = np.ascontiguousarray(xT).astype(np.float32)
    xh = xT.astype(np.float16)
    xl = (xT - xh.astype(np.float32)).astype(np.float16)
    # regroup to 2-step blocks: [NG, 128, 2, 128]
    xgh = np.ascontiguousarray(xh.reshape(128, 2, NG, 128).transpose(2, 0, 1, 3))
    xgl = np.ascontiguousarray(xl.reshape(128, 2, NG, 128).transpose(2, 0, 1, 3))

    # merged (left|right) child row indices, wrapped % 16 across partitions
    idx = np.zeros((128, S, 8), np.int16)
    lif = li[:, b0:b0 + BS] * BS + np.arange(BS)[None, :]   # [S, BS]
    rif = ri[:, b0:b0 + BS] * BS + np.arange(BS)[None, :]
    merged = np.concatenate([lif, rif], axis=1)             # [S, 128]; col j
    for j in range(128):
        idx[np.arange(128) % 16 == (j % 16), :, j // 16] = merged[:, j][None, :]

    maskv = np.ascontiguousarray(
        x_mask[:, b0:b0 + BS].reshape(1, NE)).astype(np.float16)

    return {"xgh": xgh, "xgl": xgl, "idx": idx, "maskv": maskv}


def _prep_shared_inputs(Wx, bx, Wlh, Wrh):
    # W2[zd, gk]: zd<256 -> Wlh[g,k,zd]; zd>=256 -> Wrh[g,k,zd-256]
    W2 = np.zeros((2 * D, 5 * D), np.float32)
    for g in range(5):
        W2[:D, g * D:(g + 1) * D] = Wlh[g].T
        W2[D:, g * D:(g + 1) * D] = Wrh[g].T
    w2 = np.zeros((128, 4 * NMC, 128), np.float32)
    for mc in range(NMC):
        for kc in range(4):
            w2[:, mc * 4 + kc, :] = W2[kc * 128:(kc + 1) * 128,
                                       mc * 128:(mc + 1) * 128]
    WxM = np.zeros((D, 4 * D), np.float32)
    for g in range(4):
        WxM[:, g * D:(g + 1) * D] = Wx[g].T
    wx = np.zeros((128, 16, 128), np.float32)
    for mc in range(8):
        for kc in range(2):
            wx[:, mc * 2 + kc, :] = WxM[kc * 128:(kc + 1) * 128,
                                        mc * 128:(mc + 1) * 128]
    wxh = wx.astype(np.float16)
    wxl = (wx - wxh.astype(np.float32)).astype(np.float16)
    bxf = bx.reshape(4 * D)
    bx10 = np.zeros((1, NMC, 128), np.float32)
    for pc in range(NMC):
        mc = XP_MAP10[pc]
        bx10[0, pc, :] = bxf[mc * 128:(mc + 1) * 128]
    return {"w2": w2.astype(np.float16), "wxh": wxh, "wxl": wxl,
            "bx10": bx10.astype(np.float16)}


def kernel(x, x_mask, x_left_mask, x_right_mask, Wx, bx, Wlh, Wrh):
    x = np.asarray(x, np.float32)
    x_mask = np.asarray(x_mask, np.float32)
    li = np.argmax(np.asarray(x_left_mask), axis=-1).astype(np.int64)   # [S, B]
    ri = np.argmax(np.asarray(x_right_mask), axis=-1).astype(np.int64)
    Wx = np.asarray(Wx, np.float32)
    bx = np.asarray(bx, np.float32)
    Wlh = np.asarray(Wlh, np.float32)
    Wrh = np.asarray(Wrh, np.float32)

    if "nc" not in _CACHED:
        _CACHED["nc"] = build_program()
    nc = _CACHED["nc"]

    shared = _prep_shared_inputs(Wx, bx, Wlh, Wrh)
    in_maps = []
    for core in range(NCORES):
        m = _prep_core_inputs(x, x_mask, li, ri, Wx, bx, Wlh, Wrh, core)
        m.update(shared)
        in_maps.append(m)

    res = bass_utils.run_bass_kernel_spmd(nc, in_maps, core_ids=list(range(NCORES)))
    _CACHED["last_results"] = res

    out = np.empty((B, S, D), np.float32)
    for core in range(NCORES):
        hT = np.asarray(res.results[core]["hT"]).astype(np.float32)  # [128, NE, 2]
        # out[b0+b, s, dhi*128+p] = hT[p, s*64+b, dhi]
        out[core * BS:(core + 1) * BS] = (
            hT.reshape(128, S, BS, 2).transpose(2, 1, 3, 0).reshape(BS, S, D))
    return out
